# revision 2
# baseline (speedup 1.0000x reference)
"""nn_DenseGrid trilinear embedding lookup on 8 Trainium2 cores.

Strategy (data-parallel over points, codebook replicated per core):
  - 2,097,152 points sharded 8 ways (262,144 per core); full output gathered
    on host by concatenation.
  - Per core, points are processed in super-chunks of 128*F (partition p,
    slot f). For each point: fold transform+scale into q = A@p + b, floor
    (magic-number round + fixup, no reliance on HW cast rounding mode),
    fractional weights, base row index = x + 128y + 16384z.
  - Gather: indirect DMA, one descriptor per partition per instruction
    (the only mode trn2 walrus supports): for each point-slot f and each of
    the 4 (y,z) corner combos, gather the 2-row x-pair (36 floats) with the
    corner offset applied via the instruction's element_offset constant.
  - Interpolation: G *= W8 (8 corner weights broadcast over 18 features),
    then in-place tree reduction 144 -> 72 -> 36 -> 18 per point; strided
    store back to DRAM.
"""

import numpy as np

RES = 128
FEAT = 18
V = RES**3
MAGIC = float(2**23)
P = 128
N_CORES = 8
F = 64                      # point slots per partition per super-chunk
USE_ELEM_OFFSET = True

_cache = {}


def _build(n_points, A, b):
    import concourse.bass as bass
    import concourse.bacc as bacc
    import concourse.mybir as mybir
    import concourse.tile as tile

    f32 = mybir.dt.float32
    i32 = mybir.dt.int32
    Copy = mybir.ActivationFunctionType.Copy
    Op = mybir.AluOpType

    chunk = P * F
    n_chunks = n_points // chunk
    assert n_chunks * chunk == n_points

    nc = bacc.Bacc(None, target_bir_lowering=False, debug=False)
    pts = nc.declare_dram_parameter("pts", [n_points, 3], f32, isOutput=False)
    cb = nc.declare_dram_parameter("codebook", [V, FEAT], f32, isOutput=False)
    out = nc.declare_dram_parameter("out", [n_points, FEAT], f32, isOutput=True)

    # element offsets (in table elements) of the 4 (y,z) corner combos
    corner_off = [0, RES * FEAT, RES * RES * FEAT, (RES * RES + RES) * FEAT]

    with tile.TileContext(nc) as tc:
        with (
            tc.tile_pool(name="g", bufs=2) as gpool,
            tc.tile_pool(name="small", bufs=2) as spool,
        ):
            for c in range(n_chunks):
                c0 = c * chunk
                PT = spool.tile([P, 3 * F], f32, tag="PT")
                nc.sync.dma_start(
                    out=PT[:],
                    in_=pts[c0 : c0 + chunk, :].rearrange("(p f) c -> p (f c)", p=P),
                )
                PT3 = PT[:].rearrange("p (f c) -> p f c", c=3)

                Q = spool.tile([P, 3, F], f32, tag="Q")
                FL = spool.tile([P, 3, F], f32, tag="FL")
                W = spool.tile([P, 3, F], f32, tag="W")
                U = spool.tile([P, 3, F], f32, tag="U")
                T = spool.tile([P, 3, F], f32, tag="T")
                # q_k = A[k,0]x + A[k,1]y + A[k,2]z + b_k
                for k in range(3):
                    nc.scalar.activation(Q[:, k, :], PT3[:, :, 0], Copy,
                                         bias=float(b[k]), scale=float(A[k][0]))
                    nc.scalar.activation(T[:, k, :], PT3[:, :, 1], Copy,
                                         bias=0.0, scale=float(A[k][1]))
                    nc.vector.tensor_tensor(out=Q[:, k, :], in0=Q[:, k, :], in1=T[:, k, :], op=Op.add)
                    nc.scalar.activation(T[:, k, :], PT3[:, :, 2], Copy,
                                         bias=0.0, scale=float(A[k][2]))
                    nc.vector.tensor_tensor(out=Q[:, k, :], in0=Q[:, k, :], in1=T[:, k, :], op=Op.add)
                # floor(q): round-to-nearest via magic constant, then fix up
                nc.scalar.activation(T[:], Q[:], Copy, bias=MAGIC)
                nc.scalar.activation(FL[:], T[:], Copy, bias=-MAGIC)
                nc.vector.tensor_tensor(out=T[:], in0=FL[:], in1=Q[:], op=Op.is_gt)
                nc.vector.tensor_tensor(out=FL[:], in0=FL[:], in1=T[:], op=Op.subtract)
                # frac weights (from unclipped floor), then clip floor to [0,126]
                nc.vector.tensor_tensor(out=W[:], in0=Q[:], in1=FL[:], op=Op.subtract)
                nc.vector.tensor_scalar(out=FL[:], in0=FL[:], scalar1=0.0, scalar2=float(RES - 2),
                                        op0=Op.max, op1=Op.min)
                nc.scalar.activation(U[:], W[:], Copy, bias=1.0, scale=-1.0)

                # 4 plane weights then 8 corner weights [f, c, dx]
                W4 = spool.tile([P, 4, F], f32, tag="W4")
                nc.vector.tensor_tensor(out=W4[:, 0, :], in0=U[:, 1, :], in1=U[:, 2, :], op=Op.mult)
                nc.vector.tensor_tensor(out=W4[:, 1, :], in0=W[:, 1, :], in1=U[:, 2, :], op=Op.mult)
                nc.vector.tensor_tensor(out=W4[:, 2, :], in0=U[:, 1, :], in1=W[:, 2, :], op=Op.mult)
                nc.vector.tensor_tensor(out=W4[:, 3, :], in0=W[:, 1, :], in1=W[:, 2, :], op=Op.mult)
                W8 = spool.tile([P, F, 8], f32, tag="W8")
                for cc in range(4):
                    nc.vector.tensor_tensor(out=W8[:, :, 2 * cc], in0=W4[:, cc, :], in1=U[:, 0, :], op=Op.mult)
                    nc.vector.tensor_tensor(out=W8[:, :, 2 * cc + 1], in0=W4[:, cc, :], in1=W[:, 0, :], op=Op.mult)

                # base row index = fx + 128 fy + 16384 fz  (exact in f32)
                B = spool.tile([P, F], f32, tag="B")
                T2 = spool.tile([P, 2, F], f32, tag="T2")
                nc.scalar.activation(T2[:, 0, :], FL[:, 1, :], Copy, scale=float(RES))
                nc.scalar.activation(T2[:, 1, :], FL[:, 2, :], Copy, scale=float(RES * RES))
                nc.vector.tensor_tensor(out=B[:], in0=FL[:, 0, :], in1=T2[:, 0, :], op=Op.add)
                nc.vector.tensor_tensor(out=B[:], in0=B[:], in1=T2[:, 1, :], op=Op.add)
                IDX = spool.tile([P, F, 4], i32, tag="IDX")
                nc.vector.tensor_copy(out=IDX[:, :, 0], in_=B[:])
                nc.vector.tensor_scalar(out=IDX[:, :, 1], in0=B[:], scalar1=float(RES),
                                        scalar2=None, op0=Op.add)
                nc.vector.tensor_scalar(out=IDX[:, :, 2], in0=B[:], scalar1=float(RES * RES),
                                        scalar2=None, op0=Op.add)
                nc.vector.tensor_scalar(out=IDX[:, :, 3], in0=B[:], scalar1=float(RES * RES + RES),
                                        scalar2=None, op0=Op.add)

                # gather: per point-slot f, per corner combo cc: 36 floats
                G = gpool.tile([P, F, 4, 36], f32, tag="G")
                for g in range(F):
                    for cc in range(4):
                        nc.gpsimd.indirect_dma_start(
                            out=G[:, g, cc, :],
                            out_offset=None,
                            in_=cb[:],
                            in_offset=bass.IndirectOffsetOnAxis(ap=IDX[:, g, cc : cc + 1], axis=0),
                        )

                # weighted multiply + in-place tree reduction
                Gv = G[:].rearrange("p f c e -> p (f c e)").rearrange(
                    "p (f d j) -> p f d j", d=8, j=FEAT)
                W8b = W8[:].unsqueeze(-1).broadcast_to([P, F, 8, FEAT])
                nc.vector.tensor_tensor(out=Gv, in0=Gv, in1=W8b, op=Op.mult)
                Gf = G[:].rearrange("p f c e -> p (f c e)")
                for width in (72, 36, 18):
                    a = Gf.rearrange("p (f e) -> p f e", e=144)[:, :, 0:width]
                    bb = Gf.rearrange("p (f e) -> p f e", e=144)[:, :, width : 2 * width]
                    nc.vector.tensor_tensor(out=a, in0=a, in1=bb, op=Op.add)

                res = Gf.rearrange("p (f e) -> p f e", e=144)[:, :, 0:FEAT]
                nc.sync.dma_start(
                    out=out[c0 : c0 + chunk, :].rearrange("(p f) c -> p (f c)", p=P),
                    in_=res,
                )
    nc.finalize()
    return nc


def kernel(pts, codebook, transform, _trace=False):
    from concourse.bass_utils import run_bass_kernel_spmd

    pts = np.asarray(pts, dtype=np.float32)
    codebook = np.ascontiguousarray(np.asarray(codebook, dtype=np.float32))
    transform = np.asarray(transform, dtype=np.float32)

    p_flat = np.ascontiguousarray(pts.reshape(-1, 3))
    n_total = p_flat.shape[0]
    n_per = n_total // N_CORES
    assert n_per * N_CORES == n_total

    # fold transform inverse + grid scale into affine q = A p + b (host side,
    # 4x4 input only)
    R_inv = np.linalg.inv(transform[:3, :3].astype(np.float64))
    A = (RES - 1) * R_inv
    b = -A @ transform[:3, 3].astype(np.float64)

    key = (n_per, A.tobytes(), b.tobytes())
    if key not in _cache:
        _cache[key] = _build(n_per, A, b)
    nc = _cache[key]

    in_maps = [
        {"pts": p_flat[i * n_per : (i + 1) * n_per], "codebook": codebook}
        for i in range(N_CORES)
    ]
    r = run_bass_kernel_spmd(nc, in_maps, list(range(N_CORES)), trace=_trace)
    kernel.last_exec_time_ns = r.exec_time_ns
    out = np.concatenate([r.results[i]["out"] for i in range(N_CORES)], axis=0)
    return out


kernel.last_exec_time_ns = None
